# revision 4
# baseline (speedup 1.0000x reference)
"""Fp8 per-token/per-channel quantized linear for Trainium2, 8 NeuronCores.

Computation (matches the jax reference):
    amax[m]  = max_k |x[m, k]|                       (x is bf16)
    xs[m]    = max(amax, 1e-10) / 448
    x_q      = e4m3fn_round(x / xs)                  (values up to +-448)
    out      = bf16((x_q @ W^T) * xs * w_scales) + bf16(bias)

Mapping to TRN2 hardware:
  * TRN's fp8 E4M3 saturates at +-240 (256..448 are Inf/NaN), so we quantize
    at HALF scale: x_q' = e4m3_round(x * (224/amax)) == x_q / 2 exactly (the
    fp8 grid is self-similar under powers of two), and fold the factor 2 into
    the output scale: out = psum * (amax/224) * w_scales.  The reference
    weights are already exactly fp8-representable, so casting them is lossless.
  * Sharding: row-parallel over M (8 cores x 1024 rows).  Each core quantizes
    only its own rows, and streams the full weight, transposed on host to
    [K, N] tile layout and losslessly re-encoded to fp8.
  * x_q is transposed on-chip into [K, M] layout with PE transpose matmuls
    (contraction must sit on partitions for both matmul operands).
  * Main GEMM runs in fp8 with perf_mode=DoubleRow (k=256 per matmul).

Schedule (v2): the kernel is PE-bound (DoubleRow GEMM = ~221us + transposes).
The quantization pipeline (DVE amax reduce -> scale chain -> ACT quant copy ->
PE transpose -> PSUM evict) produces one 128-row tile every ~7.5us; the PE
consumes T(mt) + GEMM(mt,nb=0) + GEMM(mt,nb=1) = ~8.6us per tile, so
interleaving two N-blocks of GEMM into the per-tile loop keeps the PE
saturated from the first transpose onward.  The remaining 6 N-blocks run as a
pure GEMM phase 2.  x loads are issued first (split in halves to cut the
first-reduce latency), weight slabs ride the same sync ring behind them, and
the ws/bias broadcasts use the scalar ring's broadcast queue.
"""

import os
import numpy as np
import ml_dtypes
from contextlib import ExitStack

import concourse.bass as bass
import concourse.bacc as bacc
import concourse.tile as tile
from concourse import mybir
from concourse.bass_utils import run_bass_kernel_spmd
from concourse.masks import make_identity

P = 128
M, K, N = 8192, 4096, 4096
NCORES = 8
M_SHARD = M // NCORES          # 1024 rows of x per core
M_TILES = M_SHARD // P         # 8
K_SUBS = K // P                # 32
K_SUPERS = K // (2 * P)        # 16 (DoubleRow consumes 256 rows of K)
KH = K // 2                    # 2048, half-tile for split loads/reduces
N_BLK = 512
N_BLKS = N // N_BLK            # 8
NB_PHASE1 = 2                  # GEMM N-blocks interleaved into the quant loop

FP8 = mybir.dt.float8e4
F32 = mybir.dt.float32
BF16 = mybir.dt.bfloat16

USE_IS_TRANSPOSE = True

_PROGRAM_CACHE = {}


def _build_program():
    nc = bacc.Bacc(None, target_bir_lowering=False)

    x_d = nc.declare_dram_parameter("x", [M_SHARD, K], BF16, isOutput=False)
    # host layout: wt[nb, p, ksub, n] = weight[nb*512 + n, ksub*128 + p],
    # losslessly re-encoded to fp8 (reference weights are fp8-round-tripped)
    wt_d = nc.declare_dram_parameter("wt", [N_BLKS, P, K_SUBS, N_BLK], FP8, isOutput=False)
    ws_d = nc.declare_dram_parameter("ws", [N], F32, isOutput=False)
    bias_d = nc.declare_dram_parameter("bias", [N], F32, isOutput=False)
    out_d = nc.declare_dram_parameter("out", [M_SHARD, N], BF16, isOutput=True)

    x_ap = x_d[:]
    wt_ap = wt_d[:]
    out_ap = out_d[:]

    with tile.TileContext(nc) as tc, ExitStack() as ctx:
        singles = ctx.enter_context(tc.tile_pool(name="singles", bufs=1))
        xpool = ctx.enter_context(tc.tile_pool(name="xpool", bufs=3))
        xqpool = ctx.enter_context(tc.tile_pool(name="xqpool", bufs=2))
        stats = ctx.enter_context(tc.tile_pool(name="stats", bufs=4))
        xspool = ctx.enter_context(tc.tile_pool(name="xspool", bufs=M_TILES))
        xqtpool = ctx.enter_context(tc.tile_pool(name="xqtpool", bufs=M_TILES))
        wpool = ctx.enter_context(tc.tile_pool(name="wpool", bufs=4))
        opool = ctx.enter_context(tc.tile_pool(name="opool", bufs=6))
        psum_tr = ctx.enter_context(tc.tile_pool(name="psum_tr", bufs=2, space="PSUM"))
        psum_mm = ctx.enter_context(tc.tile_pool(name="psum_mm", bufs=4, space="PSUM"))

        # ---- upfront DMA issue: x tiles 0-1 (halved) first, then weight
        # slabs, all on the sync ring; broadcasts ride the scalar ring's
        # broadcast queue and are HBM-read-light.
        xa_tiles = [None] * M_TILES
        xb_tiles = [None] * M_TILES

        def issue_x(mt):
            ta = xpool.tile([P, KH], BF16, tag="xa")
            nc.sync.dma_start(out=ta[:], in_=x_ap[mt * P:(mt + 1) * P, 0:KH])
            tb = xpool.tile([P, KH], BF16, tag="xb")
            nc.sync.dma_start(out=tb[:], in_=x_ap[mt * P:(mt + 1) * P, KH:K])
            xa_tiles[mt] = ta
            xb_tiles[mt] = tb

        wslab_tiles = [None] * N_BLKS

        def issue_wslab(nb):
            t = wpool.tile([P, K_SUBS, N_BLK], FP8, tag="w")
            nc.sync.dma_start(out=t[:], in_=wt_ap[nb])
            wslab_tiles[nb] = t

        issue_x(0)
        issue_wslab(0)
        issue_x(1)
        issue_wslab(1)

        ident = singles.tile([P, P], FP8)
        make_identity(nc, ident)

        ws_b = singles.tile([P, N], F32)
        nc.scalar.dma_start(
            out=ws_b[:],
            in_=bass.AP(tensor=ws_d[:].tensor, offset=0, ap=[[0, P], [1, N]]),
        )
        bias_b = singles.tile([P, N], F32)
        nc.scalar.dma_start(
            out=bias_b[:],
            in_=bass.AP(tensor=bias_d[:].tensor, offset=0, ap=[[0, P], [1, N]]),
        )

        xs_tiles = []
        xqt_tiles = []
        prev_inv_inst = None
        prev_dve_evict = None

        def epilogue(mt, nb, pm, phase1):
            # out = bf16(psum * xs[m] * ws[n]) + bias[n]; the fused
            # scalar_tensor_tensor keeps a single rounding to bf16.
            sb1 = opool.tile([P, N_BLK], BF16, tag="sb1")
            nc.vector.scalar_tensor_tensor(
                out=sb1[:], in0=pm[:], scalar=xs_tiles[mt][:],
                in1=ws_b[:, nb * N_BLK:(nb + 1) * N_BLK],
                op0=mybir.AluOpType.mult, op1=mybir.AluOpType.mult,
            )
            sb2 = opool.tile([P, N_BLK], BF16, tag="sb2")
            eng = nc.gpsimd if phase1 else nc.vector
            eng.tensor_add(sb2[:], sb1[:], bias_b[:, nb * N_BLK:(nb + 1) * N_BLK])
            nc.sync.dma_start(
                out=out_ap[mt * P:(mt + 1) * P, nb * N_BLK:(nb + 1) * N_BLK],
                in_=sb2[:],
            )

        def gemm_block(mt, nb, phase1=False):
            pm = psum_mm.tile([P, N_BLK], F32, tag="pm")
            wslab = wslab_tiles[nb]
            for j in range(K_SUPERS):
                g, jj = divmod(j, 4)
                nc.tensor.matmul(
                    out=pm[:],
                    lhsT=xqt_tiles[mt][g][:, 2 * jj:2 * jj + 2, :],
                    rhs=wslab[:, 2 * j:2 * j + 2, :],
                    start=(j == 0), stop=(j == K_SUPERS - 1),
                    perf_mode=mybir.MatmulPerfMode.DoubleRow,
                )
            epilogue(mt, nb, pm, phase1)

        # ---- phase 1: per 128-row tile: quantize, transpose, and two
        # N-blocks of GEMM to keep the PE saturated while later tiles
        # quantize.
        for mt in range(M_TILES):
            if mt + 2 < M_TILES:
                issue_x(mt + 2)
            if mt == 0:
                issue_wslab(2)
            if mt == 1:
                issue_wslab(3)

            xta, xtb = xa_tiles[mt], xb_tiles[mt]
            amax_a = stats.tile([P, 1], F32, tag="amax_a")
            reduce_inst = nc.vector.tensor_reduce(
                out=amax_a[:], in_=xta[:],
                axis=mybir.AxisListType.X, op=mybir.AluOpType.max,
                apply_absolute_value=True,
            )
            # keep the DVE from running ahead: this tile's reduce goes after
            # the previous tile's scale chain and psum evict
            if prev_inv_inst is not None:
                tile.add_dep_helper(reduce_inst.ins, prev_inv_inst.ins, sync=False,
                                    reason="stats chain before next reduce")
            if prev_dve_evict is not None:
                tile.add_dep_helper(reduce_inst.ins, prev_dve_evict.ins, sync=False,
                                    reason="evict before next reduce")
            amax_b = stats.tile([P, 1], F32, tag="amax_b")
            nc.vector.tensor_reduce(
                out=amax_b[:], in_=xtb[:],
                axis=mybir.AxisListType.X, op=mybir.AluOpType.max,
                apply_absolute_value=True,
            )
            with tc.high_priority():
                amax = stats.tile([P, 1], F32, tag="amax")
                nc.vector.tensor_max(amax[:], amax_a[:], amax_b[:])
                # xs = max(amax, eps) * (1/224); quant scale is exactly 1/xs
                xs = xspool.tile([P, 1], F32, tag="xs")
                nc.vector.tensor_scalar(
                    out=xs[:], in0=amax[:],
                    scalar1=1e-10, scalar2=1.0 / 224.0,
                    op0=mybir.AluOpType.max, op1=mybir.AluOpType.mult,
                )
                xs_tiles.append(xs)
                inv = stats.tile([P, 1], F32, tag="inv")
                prev_inv_inst = nc.vector.reciprocal(out=inv[:], in_=xs[:])

            xqa = xqpool.tile([P, KH], FP8, tag="xqa")
            nc.scalar.activation(
                out=xqa[:], in_=xta[:],
                func=mybir.ActivationFunctionType.Copy, scale=inv[:],
            )
            xqb = xqpool.tile([P, KH], FP8, tag="xqb")
            nc.scalar.activation(
                out=xqb[:], in_=xtb[:],
                func=mybir.ActivationFunctionType.Copy, scale=inv[:],
            )

            # transpose x_q into [K, M] layout via PE; evict each 8-ksub
            # group of PSUM as fp8 (3 on ACT, 1 on DVE to balance engines)
            xqt_groups = []
            for q8 in range(K_SUBS // 8):
                src = xqa if q8 < 2 else xqb
                base = 0 if q8 < 2 else 2
                xqt_g = xqtpool.tile([P, 8, P], FP8, tag=f"xqt{q8}")
                xqt_groups.append(xqt_g)
                if USE_IS_TRANSPOSE:
                    # fp8 transpose mode writes elements on a 2-byte step
                    ptr = psum_tr.tile([P, 8, 2 * P], FP8, tag="ptr")
                    ptr_view = ptr[:, :, 0:2 * P:2]
                else:
                    ptr = psum_tr.tile([P, 8, P], F32, tag="ptr")
                    ptr_view = ptr[:]
                for i in range(8):
                    ks = (q8 - base) * 8 + i
                    nc.tensor.matmul(
                        out=ptr_view[:, i, :],
                        lhsT=src[:, ks * P:(ks + 1) * P],
                        rhs=ident[:],
                        start=True, stop=True,
                        is_transpose=USE_IS_TRANSPOSE,
                    )
                if q8 == 3:
                    prev_dve_evict = nc.vector.tensor_copy(out=xqt_g[:], in_=ptr_view[:])
                else:
                    nc.scalar.copy(out=xqt_g[:], in_=ptr_view[:])
            xqt_tiles.append(xqt_groups)

            for nb in range(NB_PHASE1):
                gemm_block(mt, nb, phase1=True)

        # ---- phase 2: pure fp8 DoubleRow GEMM over the remaining N-blocks
        for nb in range(NB_PHASE1, N_BLKS):
            if nb + 2 < N_BLKS:
                issue_wslab(nb + 2)
            for mt in range(M_TILES):
                gemm_block(mt, nb)

    nc.compile()
    return nc


def _get_program():
    if "nc" not in _PROGRAM_CACHE:
        _PROGRAM_CACHE["nc"] = _build_program()
    return _PROGRAM_CACHE["nc"]


def _run_sharded(x, weight, weight_scales, bias, trace=False):
    x = np.asarray(x).astype(ml_dtypes.bfloat16, copy=False)
    weight = np.asarray(weight, dtype=np.float32)
    weight_scales = np.asarray(weight_scales, dtype=np.float32)
    bias = np.asarray(bias, dtype=np.float32)

    # host-side sharding / layout only:
    # wt[nb, p, ksub, n] = weight[nb*512 + n, ksub*128 + p], re-encoded to
    # fp8 e4m3 (lossless: the reference weights are fp8-round-tripped values)
    wt = np.ascontiguousarray(
        weight.T.reshape(K_SUBS, P, N_BLKS, N_BLK).transpose(2, 1, 0, 3)
    ).astype(ml_dtypes.float8_e4m3)
    in_maps = []
    for c in range(NCORES):
        in_maps.append({
            "x": np.ascontiguousarray(x[c * M_SHARD:(c + 1) * M_SHARD]),
            "wt": wt,
            "ws": weight_scales,
            "bias": bias,
        })

    nc = _get_program()
    res = run_bass_kernel_spmd(nc, in_maps, core_ids=list(range(NCORES)), trace=trace)
    out = np.concatenate([res.results[c]["out"] for c in range(NCORES)], axis=0)
    return out, res.exec_time_ns


def kernel(x, weight, weight_scales, bias):
    out, _ = _run_sharded(x, weight, weight_scales, bias,
                          trace=bool(os.environ.get("KERNEL_TRACE")))
    return out


# revision 9
# speedup vs baseline: 1.2224x; 1.2224x over previous
"""Fp8 per-token/per-channel quantized linear for Trainium2, 8 NeuronCores.

Computation (matches the jax reference):
    amax[m]  = max_k |x[m, k]|                       (x is bf16)
    xs[m]    = max(amax, 1e-10) / 448
    x_q      = e4m3fn_round(x / xs)                  (values up to +-448)
    out      = bf16((x_q @ W^T) * xs * w_scales) + bf16(bias)

Mapping to TRN2 hardware:
  * TRN's fp8 E4M3 saturates at +-240 (256..448 are Inf/NaN), so we quantize
    at HALF scale: x_q' = e4m3_round(x * (224/amax)) == x_q / 2 exactly (the
    fp8 grid is self-similar under powers of two), and fold the factor 2 into
    the output scale: out = psum * (amax/224) * w_scales.  The reference
    weights are already exactly fp8-representable, so casting them is lossless.
  * Sharding: row-parallel over M (8 cores x 1024 rows).  Each core quantizes
    only its own rows, and streams the full weight, transposed on host to
    [K, N] tile layout and losslessly re-encoded to fp8.
  * x_q is transposed on-chip into [K, M] layout with PE transpose matmuls
    (contraction must sit on partitions for both matmul operands).
  * Main GEMM runs in fp8 with perf_mode=DoubleRow (k=256 per matmul).

Schedule (v2): the kernel is PE-bound (DoubleRow GEMM = ~221us + transposes).
The quantization pipeline (DVE amax reduce -> scale chain -> ACT quant copy ->
PE transpose -> PSUM evict) produces one 128-row tile every ~7.5us; the PE
consumes T(mt) + GEMM(mt,nb=0) + GEMM(mt,nb=1) = ~8.6us per tile, so
interleaving two N-blocks of GEMM into the per-tile loop keeps the PE
saturated from the first transpose onward.  The remaining 6 N-blocks run as a
pure GEMM phase 2.  x loads are issued first (split in halves to cut the
first-reduce latency), weight slabs ride the same sync ring behind them, and
the ws/bias broadcasts use the scalar ring's broadcast queue.
"""

import os
import numpy as np
import ml_dtypes
from contextlib import ExitStack

import concourse.bass as bass
import concourse.bacc as bacc
import concourse.tile as tile
from concourse import mybir
from concourse.bass_utils import run_bass_kernel_spmd
from concourse.masks import make_identity

P = 128
M, K, N = 8192, 4096, 4096
NCORES = 8
M_SHARD = M // NCORES          # 1024 rows of x per core
M_TILES = M_SHARD // P         # 8
K_SUBS = K // P                # 32
K_SUPERS = K // (2 * P)        # 16 (DoubleRow consumes 256 rows of K)
KH = K // 2                    # 2048, half-tile for split loads/reduces
N_BLK = 512
N_BLKS = N // N_BLK            # 8
NB_PHASE1 = 2                  # GEMM N-blocks interleaved into the quant loop

FP8 = mybir.dt.float8e4
F32 = mybir.dt.float32
BF16 = mybir.dt.bfloat16

USE_IS_TRANSPOSE = True

_PROGRAM_CACHE = {}


def _build_program():
    nc = bacc.Bacc(None, target_bir_lowering=False)

    x_d = nc.declare_dram_parameter("x", [M_SHARD, K], BF16, isOutput=False)
    # host layout: wt[nb, p, ksub, n] = weight[nb*512 + n, ksub*128 + p],
    # losslessly re-encoded to fp8 (reference weights are fp8-round-tripped)
    wt_d = nc.declare_dram_parameter("wt", [N_BLKS, P, K_SUBS, N_BLK], FP8, isOutput=False)
    ws_d = nc.declare_dram_parameter("ws", [N], F32, isOutput=False)
    bias_d = nc.declare_dram_parameter("bias", [N], F32, isOutput=False)
    out_d = nc.declare_dram_parameter("out", [M_SHARD, N], BF16, isOutput=True)

    x_ap = x_d[:]
    wt_ap = wt_d[:]
    out_ap = out_d[:]

    with tile.TileContext(nc) as tc, ExitStack() as ctx:
        singles = ctx.enter_context(tc.tile_pool(name="singles", bufs=1))
        xpool = ctx.enter_context(tc.tile_pool(name="xpool", bufs=3))
        xqpool = ctx.enter_context(tc.tile_pool(name="xqpool", bufs=2))
        stats = ctx.enter_context(tc.tile_pool(name="stats", bufs=4))
        xspool = ctx.enter_context(tc.tile_pool(name="xspool", bufs=M_TILES))
        xqtpool = ctx.enter_context(tc.tile_pool(name="xqtpool", bufs=M_TILES))
        wpool = ctx.enter_context(tc.tile_pool(name="wpool", bufs=4))
        opool = ctx.enter_context(tc.tile_pool(name="opool", bufs=6))
        psum_tr = ctx.enter_context(tc.tile_pool(name="psum_tr", bufs=2, space="PSUM"))
        psum_mm = ctx.enter_context(tc.tile_pool(name="psum_mm", bufs=4, space="PSUM"))

        # ---- upfront DMA issue: x tiles 0-1 (halved) first, then weight
        # slabs, all on the sync ring; broadcasts ride the scalar ring's
        # broadcast queue and are HBM-read-light.
        xa_tiles = [None] * M_TILES
        xb_tiles = [None] * M_TILES

        def issue_x(mt):
            ta = xpool.tile([P, KH], BF16, tag="xa")
            nc.sync.dma_start(out=ta[:], in_=x_ap[mt * P:(mt + 1) * P, 0:KH])
            tb = xpool.tile([P, KH], BF16, tag="xb")
            nc.sync.dma_start(out=tb[:], in_=x_ap[mt * P:(mt + 1) * P, KH:K])
            xa_tiles[mt] = ta
            xb_tiles[mt] = tb

        wslab_tiles = [None] * N_BLKS

        def issue_wslab(nb):
            t = wpool.tile([P, K_SUBS, N_BLK], FP8, tag="w")
            nc.sync.dma_start(out=t[:], in_=wt_ap[nb])
            wslab_tiles[nb] = t

        issue_x(0)
        issue_wslab(0)
        issue_x(1)

        ident = singles.tile([P, P], FP8)
        make_identity(nc, ident)

        ws_b = singles.tile([P, N], F32)
        nc.scalar.dma_start(
            out=ws_b[:],
            in_=bass.AP(tensor=ws_d[:].tensor, offset=0, ap=[[0, P], [1, N]]),
        )
        bias_b = singles.tile([P, N], F32)
        nc.scalar.dma_start(
            out=bias_b[:],
            in_=bass.AP(tensor=bias_d[:].tensor, offset=0, ap=[[0, P], [1, N]]),
        )

        xs_tiles = []
        xqt_tiles = []
        prev_inv_inst = None

        def epilogue(mt, nb, pm, phase1):
            # out = bf16(psum * xs[m] * ws[n]) + bias[n]; the fused
            # scalar_tensor_tensor keeps a single rounding to bf16.
            sb1 = opool.tile([P, N_BLK], BF16, tag="sb1")
            nc.vector.scalar_tensor_tensor(
                out=sb1[:], in0=pm[:], scalar=xs_tiles[mt][:],
                in1=ws_b[:, nb * N_BLK:(nb + 1) * N_BLK],
                op0=mybir.AluOpType.mult, op1=mybir.AluOpType.mult,
            )
            sb2 = opool.tile([P, N_BLK], BF16, tag="sb2")
            eng = nc.gpsimd if phase1 else nc.vector
            eng.tensor_add(sb2[:], sb1[:], bias_b[:, nb * N_BLK:(nb + 1) * N_BLK])
            nc.sync.dma_start(
                out=out_ap[mt * P:(mt + 1) * P, nb * N_BLK:(nb + 1) * N_BLK],
                in_=sb2[:],
            )

        def gemm_block(mt, nb, phase1=False):
            pm = psum_mm.tile([P, N_BLK], F32, tag="pm")
            wslab = wslab_tiles[nb]
            for j in range(K_SUPERS):
                g, jj = divmod(j, 4)
                nc.tensor.matmul(
                    out=pm[:],
                    lhsT=xqt_tiles[mt][g][:, 2 * jj:2 * jj + 2, :],
                    rhs=wslab[:, 2 * j:2 * j + 2, :],
                    start=(j == 0), stop=(j == K_SUPERS - 1),
                    perf_mode=mybir.MatmulPerfMode.DoubleRow,
                )
            epilogue(mt, nb, pm, phase1)

        # ---- phase 1: per 128-row tile: quantize, transpose, and two
        # N-blocks of GEMM to keep the PE saturated while later tiles
        # quantize.
        for mt in range(M_TILES):
            if mt + 2 < M_TILES:
                issue_x(mt + 2)
            # stagger the 2MB slab loads so they don't starve the x stream
            if mt <= 2:
                issue_wslab(mt + 1)

            xta, xtb = xa_tiles[mt], xb_tiles[mt]
            amax_a = stats.tile([P, 1], F32, tag="amax_a")
            reduce_inst = nc.vector.tensor_reduce(
                out=amax_a[:], in_=xta[:],
                axis=mybir.AxisListType.X, op=mybir.AluOpType.max,
                apply_absolute_value=True,
            )
            # keep the DVE from scheduling this tile's reduce ahead of the
            # previous tile's tiny scale chain (which gates ACT quant)
            if prev_inv_inst is not None:
                tile.add_dep_helper(reduce_inst.ins, prev_inv_inst.ins, sync=False,
                                    reason="stats chain before next reduce")
            amax_b = stats.tile([P, 1], F32, tag="amax_b")
            nc.vector.tensor_reduce(
                out=amax_b[:], in_=xtb[:],
                axis=mybir.AxisListType.X, op=mybir.AluOpType.max,
                apply_absolute_value=True,
            )
            with tc.high_priority():
                amax = stats.tile([P, 1], F32, tag="amax")
                nc.vector.tensor_max(amax[:], amax_a[:], amax_b[:])
                # xs = max(amax, eps) * (1/224); quant scale is exactly 1/xs
                xs = xspool.tile([P, 1], F32, tag="xs")
                nc.vector.tensor_scalar(
                    out=xs[:], in0=amax[:],
                    scalar1=1e-10, scalar2=1.0 / 224.0,
                    op0=mybir.AluOpType.max, op1=mybir.AluOpType.mult,
                )
                xs_tiles.append(xs)
                inv = stats.tile([P, 1], F32, tag="inv")
                prev_inv_inst = nc.vector.reciprocal(out=inv[:], in_=xs[:])

            xqa = xqpool.tile([P, KH], FP8, tag="xqa")
            nc.scalar.activation(
                out=xqa[:], in_=xta[:],
                func=mybir.ActivationFunctionType.Copy, scale=inv[:],
            )
            xqb = xqpool.tile([P, KH], FP8, tag="xqb")
            nc.scalar.activation(
                out=xqb[:], in_=xtb[:],
                func=mybir.ActivationFunctionType.Copy, scale=inv[:],
            )

            # transpose x_q into [K, M] layout via PE; evict each 8-ksub
            # group of PSUM as fp8 (3 on ACT, 1 on DVE to balance engines)
            xqt_groups = []
            for q8 in range(K_SUBS // 8):
                src = xqa if q8 < 2 else xqb
                base = 0 if q8 < 2 else 2
                xqt_g = xqtpool.tile([P, 8, P], FP8, tag=f"xqt{q8}")
                xqt_groups.append(xqt_g)
                if USE_IS_TRANSPOSE:
                    # fp8 transpose mode writes elements on a 2-byte step
                    ptr = psum_tr.tile([P, 8, 2 * P], FP8, tag="ptr")
                    ptr_view = ptr[:, :, 0:2 * P:2]
                else:
                    ptr = psum_tr.tile([P, 8, P], F32, tag="ptr")
                    ptr_view = ptr[:]
                for i in range(8):
                    ks = (q8 - base) * 8 + i
                    nc.tensor.matmul(
                        out=ptr_view[:, i, :],
                        lhsT=src[:, ks * P:(ks + 1) * P],
                        rhs=ident[:],
                        start=True, stop=True,
                        is_transpose=USE_IS_TRANSPOSE,
                    )
                # evicts live on ACT (in queue order right after the quant
                # copies); the DVE stays clear of the transpose->GEMM path
                nc.scalar.copy(out=xqt_g[:], in_=ptr_view[:])
            xqt_tiles.append(xqt_groups)

            for nb in range(NB_PHASE1):
                gemm_block(mt, nb, phase1=True)

        # ---- phase 2: pure fp8 DoubleRow GEMM over the remaining N-blocks
        for nb in range(NB_PHASE1, N_BLKS):
            if nb + 2 < N_BLKS:
                issue_wslab(nb + 2)
            for mt in range(M_TILES):
                gemm_block(mt, nb)

    nc.compile()
    return nc


def _get_program():
    if "nc" not in _PROGRAM_CACHE:
        _PROGRAM_CACHE["nc"] = _build_program()
    return _PROGRAM_CACHE["nc"]


def _run_sharded(x, weight, weight_scales, bias, trace=False):
    x = np.asarray(x).astype(ml_dtypes.bfloat16, copy=False)
    weight = np.asarray(weight, dtype=np.float32)
    weight_scales = np.asarray(weight_scales, dtype=np.float32)
    bias = np.asarray(bias, dtype=np.float32)

    # host-side sharding / layout only:
    # wt[nb, p, ksub, n] = weight[nb*512 + n, ksub*128 + p], re-encoded to
    # fp8 e4m3 (lossless: the reference weights are fp8-round-tripped values)
    wt = np.ascontiguousarray(
        weight.T.reshape(K_SUBS, P, N_BLKS, N_BLK).transpose(2, 1, 0, 3)
    ).astype(ml_dtypes.float8_e4m3)
    in_maps = []
    for c in range(NCORES):
        in_maps.append({
            "x": np.ascontiguousarray(x[c * M_SHARD:(c + 1) * M_SHARD]),
            "wt": wt,
            "ws": weight_scales,
            "bias": bias,
        })

    nc = _get_program()
    res = run_bass_kernel_spmd(nc, in_maps, core_ids=list(range(NCORES)), trace=trace)
    out = np.concatenate([res.results[c]["out"] for c in range(NCORES)], axis=0)
    return out, res.exec_time_ns


def kernel(x, weight, weight_scales, bias):
    out, _ = _run_sharded(x, weight, weight_scales, bias,
                          trace=bool(os.environ.get("KERNEL_TRACE")))
    return out
